# revision 6
# baseline (speedup 1.0000x reference)
"""DopDense forward: relu(x @ (w * mult) + b) on 8 trn2 NeuronCores.

Key algebra: w_new = w * mult (per-column scaling) commutes with the matmul,
so out = relu((x @ w) * mult[None, :] + b).  We compute y^T tiles (units on
partitions, batch on free axis) so the per-column mult/bias become
per-partition scale/bias of a fused Relu eviction (scalar-engine activation,
or a single DVE tensor_scalar max+mult pass when b == 0).

mult is computed on device: dd[j] = sum_i |w[i,d_j] - old[i,d_j]| (vector
engine), gating logic in j-space, then a multiplicative scatter to columns
as mult = (1 + L^T lfm1) * (1 + R^T rfm1) -- left/right target columns are
each unique, and the single collision (column 0) is handled exactly by the
product.  L/R are built on device from an iota constant via is_equal.

Sharding: data-parallel over the batch axis (8192 rows/core); w, dop state
replicated.  The matmul runs in bf16 and the output returns as bf16
(rel err ~2.9e-3, measured offline against an fp64 reference), which halves
the write traffic and makes the kernel PE-bound (~55.3us of matmuls).

Schedule: the PE must never idle once warm (the HAM clock gate re-throttles
to 1.2 GHz after idle gaps).  Aux inputs (300KB) go FIRST on the two HWDGE
queues so the mult chain finishes before the first eviction; x window 0 is
loaded in k-granular pieces so the real matmul stream starts ~8us; a short
burst of dummy matmuls covers the remaining DMA wait.  Outputs drain on the
two gpsimd SWDGE rings mid-kernel, and on the low-latency HWDGE rings for
the final window to shorten the tail.
"""

import numpy as np
import ml_dtypes


def _install_ntff_shim():
    """The trimmed antenv package in this image lacks axon_hooks, which
    concourse's trace=True path imports unconditionally.  Recreate the hook
    registry (and install the ctypes NTFF hook when available) so tracing
    works whether or not the caller enables it."""
    import sys
    import types
    try:
        import antenv
        import antenv.axon_hooks  # noqa: F401
        return
    except ImportError:
        pass
    try:
        import antenv
    except ImportError:
        return
    mod = types.ModuleType("antenv.axon_hooks")
    holder = [None]
    try:
        from trn_agent_boot.trn_boot import _ntff_profile_via_ctypes
        holder[0] = _ntff_profile_via_ctypes("/opt/axon/libaxon_pjrt.so")
    except Exception:
        pass
    mod.get_axon_ntff_profile_hook = lambda: holder[0]
    mod.set_axon_ntff_profile_hook = lambda h: holder.__setitem__(0, h)
    sys.modules["antenv.axon_hooks"] = mod
    antenv.axon_hooks = mod


_install_ntff_shim()

import concourse.bass as bass
import concourse.mybir as mybir
import concourse.tile as tile
from concourse import bacc
from concourse.bass_utils import run_bass_kernel_spmd

F32 = mybir.dt.float32
BF16 = mybir.dt.bfloat16
AF = mybir.ActivationFunctionType
ALU = mybir.AluOpType
BF16_NP = np.dtype(ml_dtypes.bfloat16)

N_CORES = 8
B = 65536
NIN = 512
UNITS = 512
N_DOP = 128
SHARD = B // N_CORES          # 8192 batch rows per core
W = 512                       # batch window per psum tile (1 PSUM bank)
NWP = SHARD // W              # 16 windows per core
KC = NIN // 128               # 4 contraction chunks
CC = UNITS // 128             # 4 unit chunks
THRESHOLD = 0.0
REF_PERIOD = 2.0

# Static dopaminergic-column index math (mirrors reference.py exactly)
DOP_IDX = np.linspace(1, UNITS - 1, N_DOP, dtype=np.int32)
LEFT_OK = ~np.isin(DOP_IDX - 1, DOP_IDX)
RIGHT_OK = ~np.isin(DOP_IDX + 1, DOP_IDX)
LCOL = (DOP_IDX - 1) % UNITS
RCOL = (DOP_IDX + 1) % UNITS

LOK10 = LEFT_OK.astype(np.float32) * np.float32(10.0 / NIN)
ROK10 = RIGHT_OK.astype(np.float32) * np.float32(10.0 / NIN)

_CACHED = {}


def build_nc(all_act: bool):
    """all_act=True routes every eviction through the scalar activation
    (exact bias); all_act=False alternates ACT / single-pass DVE max+mult,
    which folds bias out entirely and is only used when b == 0."""
    if all_act in _CACHED:
        return _CACHED[all_act]
    nc = bacc.Bacc("TRN2", target_bir_lowering=False, debug=False,
                   num_swdge_queues=2)

    xt = nc.dram_tensor("xt", [NWP, 128, KC * W], BF16, kind="ExternalInput")
    # w chunks packed as [128, (k*CC+c)*128 + m] (bf16, matmul stationary)
    wkb = nc.dram_tensor("wkb", [128, KC * CC * 128], BF16, kind="ExternalInput")
    # aux vectors packed into one wide tensor (small-row DMAs are slow):
    # [:, 0:18] = per-partition vectors (lok10, rok10, indicator, batch_ctr,
    # b0..b3, lcol%128, rcol%128, Lchunkmask[4], Rchunkmask[4]),
    # [:, 18:146] = iota rows
    NV = 18
    auxs = nc.dram_tensor("auxs", [128, NV + 128], F32, kind="ExternalInput")
    # dop columns of w^T and old^T in bf16 (the |w-old| sum is insensitive
    # to bf16 rounding: per-column relative error ~0.02%)
    auxb = nc.dram_tensor("auxb", [128, 2 * NIN], BF16, kind="ExternalInput")
    yt = nc.dram_tensor("yt", [NWP, 128, CC * W], BF16, kind="ExternalOutput")

    with tile.TileContext(nc) as tc:
        with (
            tc.tile_pool(name="const", bufs=1) as const,
            tc.tile_pool(name="aux", bufs=1) as aux,
            tc.tile_pool(name="xa", bufs=6) as xpool,
            tc.tile_pool(name="ob", bufs=8) as opool,
        ):
            # ---------- input DMAs ----------
            # aux inputs lead the two HWDGE queues (they gate mult, which
            # gates every eviction); w for the matmul goes on a SWDGE ring.
            axs_sb = const.tile([128, NV + 128], F32, tag="axs")
            nc.sync.dma_start(axs_sb[:], auxs[:])
            axb_sb = const.tile([128, 2 * NIN], BF16, tag="axb")
            nc.scalar.dma_start(axb_sb[:], auxb[:])
            wk_sb = const.tile([128, KC * CC * 128], BF16, tag="wk")
            nc.gpsimd.dma_start(wk_sb[:], wkb[:])
            v_sb = axs_sb[:, 0:NV]
            io_sb = axs_sb[:, NV:NV + 128]
            wd_sb = axb_sb[:, 0:NIN]
            od_sb = axb_sb[:, NIN:2 * NIN]

            def wk_tile(k, c):
                i = k * CC + c
                return wk_sb[:, i * 128:(i + 1) * 128]

            # x windows.  Window 0 arrives in k-granular 256KB pieces so the
            # first real matmul can start as soon as piece 0 lands; window 1
            # in halves; later windows as whole 512KB DMAs alternating over
            # the two HWDGE rings (SWDGE rings carry the output stream).
            xa_tiles = {}

            def load_xa(wp):
                xa = xpool.tile([128, KC * W], BF16, tag="xa")
                if wp == 0:
                    for k in range(KC):
                        nc.sync.dma_start(xa[:, k * W:(k + 1) * W],
                                          xt[0][:, k * W:(k + 1) * W])
                elif wp == 1:
                    nc.scalar.dma_start(xa[:, :2 * W], xt[1][:, :2 * W])
                    nc.scalar.dma_start(xa[:, 2 * W:], xt[1][:, 2 * W:])
                elif wp % 2 == 0:
                    nc.sync.dma_start(xa[:], xt[wp])
                else:
                    nc.scalar.dma_start(xa[:], xt[wp])
                xa_tiles[wp] = xa

            for wp in range(4):
                load_xa(wp)

            # PE warm-up scratch: memset on the vector engine (gpsimd is
            # busy issuing the wk DMA) so dummy matmuls start immediately
            # after the init barrier and hold the HAM clock gate open.
            scr = const.tile([128, 512], BF16, tag="scr")
            nc.vector.memset(scr[:], 0.0)

            # scatter masks from iota while waiting on wd/od:
            # Lmod[j, m] = 1 iff LCOL[j] % 128 == m (chunk selection happens
            # via the masked rhs columns in the scatter matmul)
            lmod = const.tile([128, 128], BF16, tag="lmod")
            nc.vector.tensor_scalar(lmod[:], io_sb, v_sb[:, 8:9],
                                    None, op0=ALU.is_equal)
            rmod = const.tile([128, 128], BF16, tag="rmod")
            nc.vector.tensor_scalar(rmod[:], io_sb, v_sb[:, 9:10],
                                    None, op0=ALU.is_equal)

            # ---------- aux compute: dd[j] = sum_i |w[i,d_j] - old[i,d_j]| ----
            dch = aux.tile([128, NIN], F32, tag="dch")
            nc.vector.tensor_tensor(dch[:], wd_sb, od_sb, op=ALU.subtract)
            dd = const.tile([128, 1], F32, tag="dd")
            nc.vector.tensor_reduce(
                dd[:], dch[:], axis=mybir.AxisListType.X, op=ALU.add,
                apply_absolute_value=True,
            )
            # active = (dd > THRESHOLD) & ((batch_ctr - indicator) > REF_PERIOD)
            t1 = const.tile([128, 1], F32, tag="t1")
            nc.vector.tensor_tensor(t1[:], v_sb[:, 3:4], v_sb[:, 2:3],
                                    op=ALU.subtract)
            c2 = const.tile([128, 1], F32, tag="c2")
            nc.vector.tensor_scalar(c2[:], t1[:], REF_PERIOD, None, op0=ALU.is_gt)
            c1 = const.tile([128, 1], F32, tag="c1")
            nc.vector.tensor_scalar(c1[:], dd[:], THRESHOLD, None, op0=ALU.is_gt)
            av = const.tile([128, 1], F32, tag="av")
            nc.vector.tensor_tensor(av[:], c1[:], c2[:], op=ALU.mult)
            da = const.tile([128, 1], F32, tag="da")
            nc.vector.tensor_tensor(da[:], dd[:], av[:], op=ALU.mult)
            lf1 = const.tile([128, 1], F32, tag="lf1")
            nc.vector.tensor_tensor(lf1[:], da[:], v_sb[:, 0:1], op=ALU.mult)
            rf1 = const.tile([128, 1], F32, tag="rf1")
            nc.vector.tensor_tensor(rf1[:], da[:], v_sb[:, 1:2], op=ALU.mult)
            lfc = const.tile([128, CC], BF16, tag="lfc")
            nc.vector.tensor_scalar(lfc[:], v_sb[:, 10:10 + CC], lf1[:],
                                    None, op0=ALU.mult)
            rfc = const.tile([128, CC], BF16, tag="rfc")
            nc.vector.tensor_scalar(rfc[:], v_sb[:, 14:14 + CC], rf1[:],
                                    None, op0=ALU.mult)

            # PSUM: 2 banks for warm-up/scatter, 6 for the main stream.
            with (
                tc.tile_pool(name="psx", bufs=2, space="PSUM") as psaux,
                tc.tile_pool(name="ps", bufs=6, space="PSUM") as pspool,
            ):
                warm = psaux.tile([128, 512], F32, tag="aux")
                for _ in range(6):
                    nc.tensor.matmul(warm[:], scr[:, :128], scr[:],
                                     start=True, stop=True)

                multm = const.tile([128, CC], F32, tag="multm")

                def scatter_mms():
                    # additive scatters (all 4 chunks in one matmul pair),
                    # then mult = (1 + L^T lfm1) * (1 + R^T rfm1)
                    psl = psaux.tile([128, CC], F32, tag="aux")
                    nc.tensor.matmul(psl[:], lmod[:], lfc[:],
                                     start=True, stop=True)
                    psr = psaux.tile([128, CC], F32, tag="aux")
                    nc.tensor.matmul(psr[:], rmod[:], rfc[:],
                                     start=True, stop=True)
                    lsp = const.tile([128, CC], F32, tag="lsp")
                    nc.vector.tensor_scalar(lsp[:], psl[:], 1.0, None,
                                            op0=ALU.add)
                    rsp = const.tile([128, CC], F32, tag="rsp")
                    nc.vector.tensor_scalar(rsp[:], psr[:], 1.0, None,
                                            op0=ALU.add)
                    nc.vector.tensor_tensor(multm[:], lsp[:], rsp[:],
                                            op=ALU.mult)

                mult_sb = [multm[:, cc:cc + 1] for cc in range(CC)]

                # ---------- main: y^T = (w^T x^T) scaled+biased+relu ----------
                def evict_act(ps, ob, c):
                    nc.scalar.activation(
                        ob[:], ps[:], AF.Relu,
                        bias=v_sb[:, 4 + c:5 + c], scale=mult_sb[c])

                def evict_dve(ps, ob, c):
                    # relu(z*m + b) == max(z,0)*m when b==0 and m>0
                    nc.vector.tensor_scalar(
                        ob[:], ps[:], 0.0, mult_sb[c],
                        op0=ALU.max, op1=ALU.mult)

                for wp in range(NWP):
                    if wp + 4 < NWP:
                        load_xa(wp + 4)
                    xa = xa_tiles[wp]
                    last = wp == NWP - 1
                    for c in range(CC):
                        ps = pspool.tile([128, W], F32, tag="mps")
                        for k in range(KC):
                            nc.tensor.matmul(
                                ps[:], wk_tile(k, c),
                                xa[:, k * W:(k + 1) * W],
                                start=(k == 0), stop=(k == KC - 1),
                            )
                        if wp == 0 and c == 0:
                            # scatter matmuls slot in right after the first
                            # accumulation group; the aux chain is done by
                            # now, so the PE stream never stalls on it
                            scatter_mms()
                        ob = opool.tile([128, W], BF16, tag="ob")
                        if all_act or (c % 2 == 0):
                            evict_act(ps, ob, c)
                        else:
                            evict_dve(ps, ob, c)
                        # outputs drain mostly on the SWDGE ring (HWDGE
                        # rings carry the x prefetch); the last window uses
                        # the low-latency HWDGE rings to shorten the tail
                        if last:
                            eng = nc.sync if c % 2 == 0 else nc.scalar
                        elif c % 2 == 0:
                            eng = nc.gpsimd
                        else:
                            eng = nc.sync if c == 1 else nc.scalar
                        eng.dma_start(yt[wp][:, c * W:(c + 1) * W], ob[:])

    nc.compile()
    _CACHED[all_act] = nc
    return nc


LAST_RESULTS = None


def kernel(x, w, b, dop_weights_old, indicator, batch_ctr):
    global LAST_RESULTS
    x = np.asarray(x, dtype=np.float32)
    w = np.ascontiguousarray(np.asarray(w, dtype=np.float32))
    b_arr = np.asarray(b, dtype=np.float32)
    old = np.asarray(dop_weights_old, dtype=np.float32)
    ind = np.asarray(indicator, dtype=np.float32)
    bc_val = float(np.asarray(batch_ctr).item())

    nc = build_nc(all_act=bool(np.any(b_arr)))

    # replicated (per-core identical) inputs; all reshapes/gathers are pure
    # data marshaling -- every arithmetic op happens on device
    wkb = np.ascontiguousarray(
        w.reshape(KC, 128, CC, 128).transpose(1, 0, 2, 3)
    ).reshape(128, KC * CC * 128).astype(BF16_NP)
    vcols = [LOK10, ROK10, ind.astype(np.float32),
             np.full(128, bc_val, np.float32)]
    vcols += [b_arr[c * 128:(c + 1) * 128] for c in range(CC)]
    vcols += [(LCOL % 128).astype(np.float32), (RCOL % 128).astype(np.float32)]
    vcols += [(LCOL // 128 == cc).astype(np.float32) for cc in range(CC)]
    vcols += [(RCOL // 128 == cc).astype(np.float32) for cc in range(CC)]
    vecs = np.stack(vcols, axis=1).astype(np.float32)
    iot = np.broadcast_to(np.arange(128, dtype=np.float32), (128, 128))
    auxs = np.ascontiguousarray(np.concatenate(
        [vecs, iot], axis=1, dtype=np.float32))
    auxb = np.ascontiguousarray(np.concatenate(
        [w.T[DOP_IDX], old.T[DOP_IDX]], axis=1, dtype=np.float32)
    ).astype(BF16_NP)

    common = dict(wkb=wkb, auxs=auxs, auxb=auxb)

    xbf = x.astype(BF16_NP)
    in_maps = []
    for i in range(N_CORES):
        xs = xbf[i * SHARD:(i + 1) * SHARD]          # [8192, 512]
        xtc = np.ascontiguousarray(
            xs.reshape(NWP, W, KC, 128).transpose(0, 3, 2, 1)
        ).reshape(NWP, 128, KC * W)
        in_maps.append(dict(common, xt=xtc))

    res = run_bass_kernel_spmd(nc, in_maps, core_ids=list(range(N_CORES)))
    LAST_RESULTS = res

    out = np.empty((B, UNITS), np.float32)
    for i in range(N_CORES):
        ytc = res.results[i]["yt"].reshape(NWP, 128, CC, W)
        out[i * SHARD:(i + 1) * SHARD] = (
            ytc.transpose(0, 3, 2, 1).reshape(SHARD, UNITS).astype(np.float32))
    return out
